# revision 17
# baseline (speedup 1.0000x reference)
"""MinGRU (B=4, T=4096, D=1024) Trainium2 kernel, 8-core SPMD.

Sharding: core i handles (batch b = i//2, output-channel half j = i%2).
Each core computes u_z = x[b] @ Wz[half].T, u_h = x[b] @ Wh[half].T in bf16
(PE at 1 col/cycle; FWL-eligible weight loads), then per 128-channel group:
z = sigmoid(u_z + bz), a = sigmoid(-(u_z + bz)) = 1 - z, b = z*(u_h + bh),
and the recurrence h_t = a_t*h_{t-1} + b_t via the hardware
tensor_tensor_scan (fp32 PSUM/epilogue, so only the matmul inputs are bf16).

Host pre-transposes x and W (and casts to bf16) so every DMA is
row-contiguous, and re-transposes the per-core (512, 4096) results into the
full output.  All DMAs ride the SP HWDGE queue in explicit priority order
(h-store dma_starts on the ACT queue would head-of-line-block the epilogue
ACTs).  The benchmark loop uses staggered_reset so consecutive iterations
overlap instead of paying the all-engine back-edge barrier.
"""

import numpy as np

_B, _T, _D = 4, 4096, 1024
_EH = 512          # output channels per core
_NG = _EH // 128   # 4 channel groups of 128 partitions
_TT = 512          # timestep tile (one PSUM bank, max moving free dim)
_NT = _T // _TT    # 8 t-tiles
_NK = _D // 128    # 8 contraction tiles


def _build(reps=1, loop_n=None, staggered=True, _LASTW=256):
    from contextlib import ExitStack
    from concourse import bacc, mybir, tile

    f32 = mybir.dt.float32
    bf16 = mybir.dt.bfloat16
    AF = mybir.ActivationFunctionType
    OP = mybir.AluOpType

    nc = bacc.Bacc("TRN2", debug=False, num_devices=8)
    xt = nc.dram_tensor("xt", [_D, _T], bf16, kind="ExternalInput").ap()
    wzt = nc.dram_tensor("wzt", [_D, _EH], bf16, kind="ExternalInput").ap()
    wht = nc.dram_tensor("wht", [_D, _EH], bf16, kind="ExternalInput").ap()
    bzt = nc.dram_tensor("bzt", [128, _NG], f32, kind="ExternalInput").ap()
    nbzt = nc.dram_tensor("nbzt", [128, _NG], f32, kind="ExternalInput").ap()
    bht = nc.dram_tensor("bht", [128, _NG], f32, kind="ExternalInput").ap()
    hout = nc.dram_tensor("h", [_EH, _T], f32, kind="ExternalOutput").ap()

    with tile.TileContext(nc) as tc, ExitStack() as ctx:
        wpool = ctx.enter_context(tc.tile_pool(name="w", bufs=1))
        xpool = ctx.enter_context(tc.tile_pool(name="x", bufs=3))
        vpool = ctx.enter_context(tc.tile_pool(name="v", bufs=3))
        hpool = ctx.enter_context(tc.tile_pool(name="h", bufs=2))
        ppool = ctx.enter_context(tc.tile_pool(name="p", bufs=4, space="PSUM"))

        def load_x(t, n_chunks=2):
            # x^T tile for this t-slice: [128, (k tt)], chunked k-block DMAs.
            xs = xpool.tile([128, _NK * _TT], bf16, tag="x")
            step = _NK // n_chunks
            for c in range(n_chunks):
                ks = c * step
                nc.sync.dma_start(
                    xs[:, ks * _TT:(ks + step) * _TT].rearrange(
                        "p (k t) -> p k t", k=step),
                    xt.rearrange("(k p) t -> p k t", p=128)[
                        :, ks:ks + step, t * _TT:(t + 1) * _TT],
                )
            return xs

        # Startup DMA order matters: the first matmuls need x chunk0 and the
        # first weight k-tiles; interleave x chunks with per-k weight tiles
        # so PE starts ~2us in and stays fed while the rest streams.
        xs0 = xpool.tile([128, _NK * _TT], bf16, tag="x")
        wz_sb = wpool.tile([128, _NK * _EH], bf16, tag="wz")
        wh_sb = wpool.tile([128, _NK * _EH], bf16, tag="wh")
        bz_sb = wpool.tile([128, _NG], f32, tag="bz")
        nbz_sb = wpool.tile([128, _NG], f32, tag="nbz")
        bh_sb = wpool.tile([128, _NG], f32, tag="bh")

        def x0_chunk(ks, nk, eng):
            eng.dma_start(
                xs0[:, ks * _TT:(ks + nk) * _TT].rearrange(
                    "p (k t) -> p k t", k=nk),
                xt.rearrange("(k p) t -> p k t", p=128)[
                    :, ks:ks + nk, 0:_TT],
            )

        def w_chunk(k, eng):
            eng.dma_start(
                wz_sb[:, k * _EH:(k + 1) * _EH],
                wzt[k * 128:(k + 1) * 128, :],
            )
            eng.dma_start(
                wh_sb[:, k * _EH:(k + 1) * _EH],
                wht[k * 128:(k + 1) * 128, :],
            )

        # Startup rides ONE queue (sync) in consumption order: the two HWDGE
        # queues round-robin on shared HBM bandwidth, so split ordering lets
        # x chunks starve the per-k weight tiles the t=0 matmuls (k-major)
        # need.  x for t=1 is prefetched here too, between late weight tiles.
        w_chunk(0, nc.sync)
        x0_chunk(0, 2, nc.sync)
        w_chunk(1, nc.sync)
        nc.sync.dma_start(bz_sb[:], bzt)
        nc.sync.dma_start(nbz_sb[:], nbzt)
        nc.sync.dma_start(bh_sb[:], bht)
        x0_chunk(2, 2, nc.sync)
        w_chunk(2, nc.sync)
        w_chunk(3, nc.sync)
        x0_chunk(4, 2, nc.sync)
        w_chunk(4, nc.sync)
        x0_chunk(6, 2, nc.sync)
        w_chunk(5, nc.sync)
        w_chunk(6, nc.sync)
        w_chunk(7, nc.sync)
        xs1 = load_x(1)

        def body(first, in_loop):
          hprev = [None] * _NG
          for t in range(_NT):
            if in_loop and staggered and t in (2, 4, 6):
                tc.stage_boundary()
            if first and t == 0:
                xs = xs0
            elif first and t == 1:
                xs = xs1
            else:
                xs = load_x(t)
            for g in range(_NG):
                last = (t == _NT - 1 and g == _NG - 1)
                prev_ap = None if t == 0 else hprev[g][:, _TT - 1:_TT]
                pz = ppool.tile([128, _TT], f32, tag="pz")
                ph = ppool.tile([128, _TT], f32, tag="ph")
                for k in range(_NK):
                    nc.tensor.matmul(
                        pz[:],
                        lhsT=wz_sb[:, k * _EH + g * 128: k * _EH + (g + 1) * 128],
                        rhs=xs[:, k * _TT:(k + 1) * _TT],
                        start=(k == 0),
                        stop=(k == _NK - 1),
                    )
                for k in range(_NK):
                    nc.tensor.matmul(
                        ph[:],
                        lhsT=wh_sb[:, k * _EH + g * 128: k * _EH + (g + 1) * 128],
                        rhs=xs[:, k * _TT:(k + 1) * _TT],
                        start=(k == 0),
                        stop=(k == _NK - 1),
                    )
                # Final group drains its epilogue in 128-col slices so the
                # ACT/DVE/DMA chain overlaps itself instead of running
                # full-width after the last matmul.
                slices = tuple((c, _LASTW) for c in range(0, _TT, _LASTW)) \
                    if last else ((0, _TT),)
                hb = hpool.tile([128, _TT], f32, tag=f"h{g}")
                for (c0, w) in slices:
                    z = vpool.tile([128, w], f32, tag="z")
                    nc.scalar.activation(z[:], pz[:, c0:c0 + w], AF.Sigmoid,
                                         bias=bz_sb[:, g:g + 1])
                    # a = 1 - z == sigmoid(-(u+bz)); independent of z's ACT.
                    av = vpool.tile([128, w], f32, tag="a")
                    nc.scalar.activation(av[:], pz[:, c0:c0 + w], AF.Sigmoid,
                                         bias=nbz_sb[:, g:g + 1], scale=-1.0)
                    bv = vpool.tile([128, w], f32, tag="b")
                    nc.vector.scalar_tensor_tensor(
                        bv[:], ph[:, c0:c0 + w], bh_sb[:, g:g + 1], z[:],
                        OP.add, OP.mult
                    )
                    init = 0.0 if prev_ap is None else prev_ap
                    nc.vector.tensor_tensor_scan(hb[:, c0:c0 + w], av[:],
                                                 bv[:], init,
                                                 OP.mult, OP.add)
                    prev_ap = hb[:, c0 + w - 1:c0 + w]
                    nc.sync.dma_start(
                        hout[g * 128:(g + 1) * 128,
                             t * _TT + c0: t * _TT + c0 + w],
                        hb[:, c0:c0 + w]
                    )
                hprev[g] = hb

        if loop_n is not None:
            body(True, False)
            from concourse import mybir as _mb
            with tc.For_i(0, loop_n, 1,
                          hint_engines=(_mb.EngineType.PE, _mb.EngineType.SP,
                                        _mb.EngineType.DVE,
                                        _mb.EngineType.Activation),
                          staggered_reset=staggered):
                body(False, True)
        else:
            for rep in range(reps):
                body(rep == 0, False)
    nc.compile()
    return nc


_NC_CACHE = None


def _shard_inputs(inputs):
    """Per-core input maps: host pre-transposes so device DMAs are contiguous."""
    import ml_dtypes
    bf16 = ml_dtypes.bfloat16

    x = np.asarray(inputs["x"], dtype=np.float32)
    Wz = np.asarray(inputs["Wz"], dtype=np.float32)
    bz = np.asarray(inputs["bz"], dtype=np.float32)
    Wh = np.asarray(inputs["Wh"], dtype=np.float32)
    bh = np.asarray(inputs["bh"], dtype=np.float32)

    wzT = np.ascontiguousarray(Wz.T).astype(bf16)  # [d, e]
    whT = np.ascontiguousarray(Wh.T).astype(bf16)

    in_maps = []
    for i in range(8):
        b, j = i // 2, i % 2
        sl = slice(j * _EH, (j + 1) * _EH)
        in_maps.append({
            "xt": np.ascontiguousarray(x[b].T).astype(bf16),  # [D, T]
            "wzt": np.ascontiguousarray(wzT[:, sl]),          # [D, EH]
            "wht": np.ascontiguousarray(whT[:, sl]),          # [D, EH]
            "bzt": np.ascontiguousarray(bz[sl].reshape(_NG, 128).T),  # [128, NG]
            "nbzt": np.ascontiguousarray((-bz[sl]).reshape(_NG, 128).T),
            "bht": np.ascontiguousarray(bh[sl].reshape(_NG, 128).T),
        })
    return in_maps


def run(inputs, trace=False, tmpdir=None):
    """Shard inputs, run the SPMD kernel on 8 cores, reassemble full output.

    Returns (output ndarray (B, T, D) float32, BassKernelResults).
    """
    global _NC_CACHE
    from concourse.bass_utils import run_bass_kernel_spmd

    if _NC_CACHE is None:
        _NC_CACHE = _build()
    nc = _NC_CACHE

    in_maps = _shard_inputs(inputs)

    res = run_bass_kernel_spmd(
        nc, in_maps, core_ids=list(range(8)), trace=trace, tmpdir=tmpdir
    )

    out = np.empty((_B, _T, _D), dtype=np.float32)
    for i in range(8):
        b, j = i // 2, i % 2
        out[b, :, j * _EH:(j + 1) * _EH] = res.results[i]["h"].T
    return out, res


def kernel(**inputs):
    out, _ = run(inputs, trace=False)
    return out


# revision 18
# speedup vs baseline: 1.4147x; 1.4147x over previous
"""MinGRU (B=4, T=4096, D=1024) Trainium2 kernel, 8-core SPMD.

Sharding: core i handles (batch b = i//2, output-channel half j = i%2).
Each core computes u_z = x[b] @ Wz[half].T, u_h = x[b] @ Wh[half].T in bf16
(PE at 1 col/cycle; FWL-eligible weight loads), then per 128-channel group:
z = sigmoid(u_z + bz), a = sigmoid(-(u_z + bz)) = 1 - z, b = z*(u_h + bh),
and the recurrence h_t = a_t*h_{t-1} + b_t via the hardware
tensor_tensor_scan (fp32 PSUM/epilogue, so only the matmul inputs are bf16).

Host pre-transposes x and W (and casts to bf16) so every DMA is
row-contiguous, and re-transposes the per-core (512, 4096) results into the
full output.  All DMAs ride the SP HWDGE queue in explicit priority order
(h-store dma_starts on the ACT queue would head-of-line-block the epilogue
ACTs).  The benchmark loop uses staggered_reset so consecutive iterations
overlap instead of paying the all-engine back-edge barrier.
"""

import numpy as np

_B, _T, _D = 4, 4096, 1024
_EH = 512          # output channels per core
_NG = _EH // 128   # 4 channel groups of 128 partitions
_TT = 512          # timestep tile (one PSUM bank, max moving free dim)
_NT = _T // _TT    # 8 t-tiles
_NK = _D // 128    # 8 contraction tiles


def _build(reps=1, loop_n=None, staggered=False, _LASTW=256):
    from contextlib import ExitStack
    from concourse import bacc, mybir, tile

    f32 = mybir.dt.float32
    bf16 = mybir.dt.bfloat16
    AF = mybir.ActivationFunctionType
    OP = mybir.AluOpType

    nc = bacc.Bacc("TRN2", debug=False, num_devices=8)
    xt = nc.dram_tensor("xt", [_D, _T], bf16, kind="ExternalInput").ap()
    wzt = nc.dram_tensor("wzt", [_D, _EH], bf16, kind="ExternalInput").ap()
    wht = nc.dram_tensor("wht", [_D, _EH], bf16, kind="ExternalInput").ap()
    bzt = nc.dram_tensor("bzt", [128, _NG], f32, kind="ExternalInput").ap()
    nbzt = nc.dram_tensor("nbzt", [128, _NG], f32, kind="ExternalInput").ap()
    bht = nc.dram_tensor("bht", [128, _NG], f32, kind="ExternalInput").ap()
    hout = nc.dram_tensor("h", [_EH, _T], f32, kind="ExternalOutput").ap()

    with tile.TileContext(nc) as tc, ExitStack() as ctx:
        wpool = ctx.enter_context(tc.tile_pool(name="w", bufs=1))
        xpool = ctx.enter_context(tc.tile_pool(name="x", bufs=3))
        vpool = ctx.enter_context(tc.tile_pool(name="v", bufs=3))
        hpool = ctx.enter_context(tc.tile_pool(name="h", bufs=2))
        ppool = ctx.enter_context(tc.tile_pool(name="p", bufs=4, space="PSUM"))

        def load_x(t, n_chunks=2):
            # x^T tile for this t-slice: [128, (k tt)], chunked k-block DMAs.
            xs = xpool.tile([128, _NK * _TT], bf16, tag="x")
            step = _NK // n_chunks
            for c in range(n_chunks):
                ks = c * step
                nc.sync.dma_start(
                    xs[:, ks * _TT:(ks + step) * _TT].rearrange(
                        "p (k t) -> p k t", k=step),
                    xt.rearrange("(k p) t -> p k t", p=128)[
                        :, ks:ks + step, t * _TT:(t + 1) * _TT],
                )
            return xs

        # Startup DMA order matters: the first matmuls need x chunk0 and the
        # first weight k-tiles; interleave x chunks with per-k weight tiles
        # so PE starts ~2us in and stays fed while the rest streams.
        xs0 = xpool.tile([128, _NK * _TT], bf16, tag="x")
        wz_sb = wpool.tile([128, _NK * _EH], bf16, tag="wz")
        wh_sb = wpool.tile([128, _NK * _EH], bf16, tag="wh")
        bz_sb = wpool.tile([128, _NG], f32, tag="bz")
        nbz_sb = wpool.tile([128, _NG], f32, tag="nbz")
        bh_sb = wpool.tile([128, _NG], f32, tag="bh")

        def x0_chunk(ks, nk, eng):
            eng.dma_start(
                xs0[:, ks * _TT:(ks + nk) * _TT].rearrange(
                    "p (k t) -> p k t", k=nk),
                xt.rearrange("(k p) t -> p k t", p=128)[
                    :, ks:ks + nk, 0:_TT],
            )

        def w_chunk(k, eng):
            eng.dma_start(
                wz_sb[:, k * _EH:(k + 1) * _EH],
                wzt[k * 128:(k + 1) * 128, :],
            )
            eng.dma_start(
                wh_sb[:, k * _EH:(k + 1) * _EH],
                wht[k * 128:(k + 1) * 128, :],
            )

        # Startup rides ONE queue (sync) in consumption order: the two HWDGE
        # queues round-robin on shared HBM bandwidth, so split ordering lets
        # x chunks starve the per-k weight tiles the t=0 matmuls (k-major)
        # need.  x for t=1 is prefetched here too, between late weight tiles.
        w_chunk(0, nc.sync)
        x0_chunk(0, 2, nc.sync)
        w_chunk(1, nc.sync)
        nc.sync.dma_start(bz_sb[:], bzt)
        nc.sync.dma_start(nbz_sb[:], nbzt)
        nc.sync.dma_start(bh_sb[:], bht)
        x0_chunk(2, 2, nc.sync)
        w_chunk(2, nc.sync)
        w_chunk(3, nc.sync)
        x0_chunk(4, 2, nc.sync)
        w_chunk(4, nc.sync)
        x0_chunk(6, 2, nc.sync)
        w_chunk(5, nc.sync)
        w_chunk(6, nc.sync)
        w_chunk(7, nc.sync)
        xs1 = load_x(1)

        def body(first, in_loop):
          hprev = [None] * _NG
          for t in range(_NT):
            if in_loop and staggered and t in (2, 4, 6):
                tc.stage_boundary()
            if first and t == 0:
                xs = xs0
            elif first and t == 1:
                xs = xs1
            else:
                xs = load_x(t)
            for g in range(_NG):
                last = (t == _NT - 1 and g == _NG - 1)
                prev_ap = None if t == 0 else hprev[g][:, _TT - 1:_TT]
                pz = ppool.tile([128, _TT], f32, tag="pz")
                ph = ppool.tile([128, _TT], f32, tag="ph")
                for k in range(_NK):
                    nc.tensor.matmul(
                        pz[:],
                        lhsT=wz_sb[:, k * _EH + g * 128: k * _EH + (g + 1) * 128],
                        rhs=xs[:, k * _TT:(k + 1) * _TT],
                        start=(k == 0),
                        stop=(k == _NK - 1),
                    )
                for k in range(_NK):
                    nc.tensor.matmul(
                        ph[:],
                        lhsT=wh_sb[:, k * _EH + g * 128: k * _EH + (g + 1) * 128],
                        rhs=xs[:, k * _TT:(k + 1) * _TT],
                        start=(k == 0),
                        stop=(k == _NK - 1),
                    )
                # Final group drains its epilogue in 128-col slices so the
                # ACT/DVE/DMA chain overlaps itself instead of running
                # full-width after the last matmul.
                slices = tuple((c, _LASTW) for c in range(0, _TT, _LASTW)) \
                    if last else ((0, _TT),)
                hb = hpool.tile([128, _TT], f32, tag=f"h{g}")
                for (c0, w) in slices:
                    z = vpool.tile([128, w], f32, tag="z")
                    nc.scalar.activation(z[:], pz[:, c0:c0 + w], AF.Sigmoid,
                                         bias=bz_sb[:, g:g + 1])
                    # a = 1 - z == sigmoid(-(u+bz)); independent of z's ACT.
                    av = vpool.tile([128, w], f32, tag="a")
                    nc.scalar.activation(av[:], pz[:, c0:c0 + w], AF.Sigmoid,
                                         bias=nbz_sb[:, g:g + 1], scale=-1.0)
                    bv = vpool.tile([128, w], f32, tag="b")
                    nc.vector.scalar_tensor_tensor(
                        bv[:], ph[:, c0:c0 + w], bh_sb[:, g:g + 1], z[:],
                        OP.add, OP.mult
                    )
                    init = 0.0 if prev_ap is None else prev_ap
                    nc.vector.tensor_tensor_scan(hb[:, c0:c0 + w], av[:],
                                                 bv[:], init,
                                                 OP.mult, OP.add)
                    prev_ap = hb[:, c0 + w - 1:c0 + w]
                    nc.sync.dma_start(
                        hout[g * 128:(g + 1) * 128,
                             t * _TT + c0: t * _TT + c0 + w],
                        hb[:, c0:c0 + w]
                    )
                hprev[g] = hb

        if loop_n is not None:
            body(True, False)
            from concourse import mybir as _mb
            with tc.For_i(0, loop_n, 1,
                          hint_engines=(_mb.EngineType.PE, _mb.EngineType.SP,
                                        _mb.EngineType.DVE,
                                        _mb.EngineType.Activation),
                          staggered_reset=staggered):
                body(False, True)
        else:
            for rep in range(reps):
                body(rep == 0, False)
    nc.compile()
    return nc


_NC_CACHE = None


def _shard_inputs(inputs):
    """Per-core input maps: host pre-transposes so device DMAs are contiguous."""
    import ml_dtypes
    bf16 = ml_dtypes.bfloat16

    x = np.asarray(inputs["x"], dtype=np.float32)
    Wz = np.asarray(inputs["Wz"], dtype=np.float32)
    bz = np.asarray(inputs["bz"], dtype=np.float32)
    Wh = np.asarray(inputs["Wh"], dtype=np.float32)
    bh = np.asarray(inputs["bh"], dtype=np.float32)

    wzT = np.ascontiguousarray(Wz.T).astype(bf16)  # [d, e]
    whT = np.ascontiguousarray(Wh.T).astype(bf16)

    in_maps = []
    for i in range(8):
        b, j = i // 2, i % 2
        sl = slice(j * _EH, (j + 1) * _EH)
        in_maps.append({
            "xt": np.ascontiguousarray(x[b].T).astype(bf16),  # [D, T]
            "wzt": np.ascontiguousarray(wzT[:, sl]),          # [D, EH]
            "wht": np.ascontiguousarray(whT[:, sl]),          # [D, EH]
            "bzt": np.ascontiguousarray(bz[sl].reshape(_NG, 128).T),  # [128, NG]
            "nbzt": np.ascontiguousarray((-bz[sl]).reshape(_NG, 128).T),
            "bht": np.ascontiguousarray(bh[sl].reshape(_NG, 128).T),
        })
    return in_maps


def run(inputs, trace=False, tmpdir=None):
    """Shard inputs, run the SPMD kernel on 8 cores, reassemble full output.

    Returns (output ndarray (B, T, D) float32, BassKernelResults).
    """
    global _NC_CACHE
    from concourse.bass_utils import run_bass_kernel_spmd

    if _NC_CACHE is None:
        _NC_CACHE = _build()
    nc = _NC_CACHE

    in_maps = _shard_inputs(inputs)

    res = run_bass_kernel_spmd(
        nc, in_maps, core_ids=list(range(8)), trace=trace, tmpdir=tmpdir
    )

    out = np.empty((_B, _T, _D), dtype=np.float32)
    for i in range(8):
        b, j = i // 2, i % 2
        out[b, :, j * _EH:(j + 1) * _EH] = res.results[i]["h"].T
    return out, res


def kernel(**inputs):
    out, _ = run(inputs, trace=False)
    return out


# revision 23
# speedup vs baseline: 1.4524x; 1.0266x over previous
"""MinGRU (B=4, T=4096, D=1024) Trainium2 kernel, 8-core SPMD.

Sharding: core i handles (batch b = i//2, output-channel half j = i%2).
Each core computes u_z = x[b] @ Wz[half].T, u_h = x[b] @ Wh[half].T in bf16
(PE at 1 col/cycle; FWL-eligible weight loads), then per 128-channel group:
z = sigmoid(u_z + bz), a = sigmoid(-(u_z + bz)) = 1 - z, b = z*(u_h + bh),
and the recurrence h_t = a_t*h_{t-1} + b_t via the hardware
tensor_tensor_scan (fp32 PSUM/epilogue, so only the matmul inputs are bf16).

Host pre-transposes x and W (and casts to bf16) so every DMA is
row-contiguous, and re-transposes the per-core (512, 4096) results into the
full output.  All DMAs ride the SP HWDGE queue in explicit priority order
(h-store dma_starts on the ACT queue would head-of-line-block the epilogue
ACTs; measured on HW: staggered_reset loop pipelining is a big net loss, so
the loop keeps the plain barrier back-edge, amortized via unroll).
"""

import numpy as np

_B, _T, _D = 4, 4096, 1024
_EH = 512          # output channels per core
_NG = _EH // 128   # 4 channel groups of 128 partitions
_TT = 512          # timestep tile (one PSUM bank, max moving free dim)
_NT = _T // _TT    # 8 t-tiles
_NK = _D // 128    # 8 contraction tiles


def _build(reps=1, loop_n=None, staggered=False, _LASTW=256,
           av_from_z=True, psum_bufs=3, split_last=True, unroll=2):
    from contextlib import ExitStack
    from concourse import bacc, mybir, tile

    f32 = mybir.dt.float32
    bf16 = mybir.dt.bfloat16
    AF = mybir.ActivationFunctionType
    OP = mybir.AluOpType

    nc = bacc.Bacc("TRN2", debug=False, num_devices=8)
    xt = nc.dram_tensor("xt", [_D, _T], bf16, kind="ExternalInput").ap()
    wzt = nc.dram_tensor("wzt", [_D, _EH], bf16, kind="ExternalInput").ap()
    wht = nc.dram_tensor("wht", [_D, _EH], bf16, kind="ExternalInput").ap()
    bzt = nc.dram_tensor("bzt", [128, _NG], f32, kind="ExternalInput").ap()
    nbzt = nc.dram_tensor("nbzt", [128, _NG], f32, kind="ExternalInput").ap()
    bht = nc.dram_tensor("bht", [128, _NG], f32, kind="ExternalInput").ap()
    hout = nc.dram_tensor("h", [_EH, _T], f32, kind="ExternalOutput").ap()

    with tile.TileContext(nc) as tc, ExitStack() as ctx:
        wpool = ctx.enter_context(tc.tile_pool(name="w", bufs=1))
        xpool = ctx.enter_context(tc.tile_pool(name="x", bufs=3))
        vpool = ctx.enter_context(tc.tile_pool(name="v", bufs=3))
        hpool = ctx.enter_context(tc.tile_pool(name="h", bufs=2))
        ppool = ctx.enter_context(
            tc.tile_pool(name="p", bufs=psum_bufs, space="PSUM"))
        plast = ctx.enter_context(
            tc.tile_pool(name="pl", bufs=1, space="PSUM")) if split_last \
            else None

        def load_x(t, n_chunks=2):
            # x^T tile for this t-slice: [128, (k tt)], chunked k-block DMAs.
            xs = xpool.tile([128, _NK * _TT], bf16, tag="x")
            step = _NK // n_chunks
            for c in range(n_chunks):
                ks = c * step
                nc.sync.dma_start(
                    xs[:, ks * _TT:(ks + step) * _TT].rearrange(
                        "p (k t) -> p k t", k=step),
                    xt.rearrange("(k p) t -> p k t", p=128)[
                        :, ks:ks + step, t * _TT:(t + 1) * _TT],
                )
            return xs

        # Startup DMA order matters: the first matmuls need x chunk0 and the
        # first weight k-tiles; interleave x chunks with per-k weight tiles
        # so PE starts ~2us in and stays fed while the rest streams.
        xs0 = xpool.tile([128, _NK * _TT], bf16, tag="x")
        wz_sb = wpool.tile([128, _NK * _EH], bf16, tag="wz")
        wh_sb = wpool.tile([128, _NK * _EH], bf16, tag="wh")
        bz_sb = wpool.tile([128, _NG], f32, tag="bz")
        nbz_sb = wpool.tile([128, _NG], f32, tag="nbz")
        bh_sb = wpool.tile([128, _NG], f32, tag="bh")

        def x0_chunk(ks, nk, eng):
            eng.dma_start(
                xs0[:, ks * _TT:(ks + nk) * _TT].rearrange(
                    "p (k t) -> p k t", k=nk),
                xt.rearrange("(k p) t -> p k t", p=128)[
                    :, ks:ks + nk, 0:_TT],
            )

        def w_chunk(k, eng):
            eng.dma_start(
                wz_sb[:, k * _EH:(k + 1) * _EH],
                wzt[k * 128:(k + 1) * 128, :],
            )
            eng.dma_start(
                wh_sb[:, k * _EH:(k + 1) * _EH],
                wht[k * 128:(k + 1) * 128, :],
            )

        # Startup rides ONE queue (sync) in consumption order: the two HWDGE
        # queues round-robin on shared HBM bandwidth, so split ordering lets
        # x chunks starve the per-k weight tiles the t=0 matmuls (k-major)
        # need.  x for t=1 is prefetched here too, between late weight tiles.
        w_chunk(0, nc.sync)
        x0_chunk(0, 2, nc.sync)
        w_chunk(1, nc.sync)
        nc.sync.dma_start(bz_sb[:], bzt)
        nc.sync.dma_start(nbz_sb[:], nbzt)
        nc.sync.dma_start(bh_sb[:], bht)
        x0_chunk(2, 2, nc.sync)
        w_chunk(2, nc.sync)
        w_chunk(3, nc.sync)
        x0_chunk(4, 2, nc.sync)
        w_chunk(4, nc.sync)
        x0_chunk(6, 2, nc.sync)
        w_chunk(5, nc.sync)
        w_chunk(6, nc.sync)
        w_chunk(7, nc.sync)
        xs1 = load_x(1)

        def epilogue(pz, ph, hb, g, t, c0, w, prev_ap):
            z = vpool.tile([128, w], f32, tag="z")
            nc.scalar.activation(z[:], pz[:, c0 - c0:c0 - c0 + w] if hb is None
                                 else pz[:, c0:c0 + w], AF.Sigmoid,
                                 bias=bz_sb[:, g:g + 1])
            av = vpool.tile([128, w], f32, tag="a")
            if av_from_z:
                # old chain: a = (1+eps) - z on ACT Copy
                nc.scalar.activation(av[:], z[:], AF.Copy,
                                     bias=1.0 + 1e-8, scale=-1.0)
            else:
                # a = 1 - z == sigmoid(-(u+bz)); independent of z's ACT.
                nc.scalar.activation(av[:], pz[:, c0:c0 + w] if hb is not None
                                     else pz[:], AF.Sigmoid,
                                     bias=nbz_sb[:, g:g + 1], scale=-1.0)
            bv = vpool.tile([128, w], f32, tag="b")
            nc.vector.scalar_tensor_tensor(
                bv[:], ph[:, c0:c0 + w] if hb is not None else ph[:],
                bh_sb[:, g:g + 1], z[:], OP.add, OP.mult)
            out_ap = hb[:, c0:c0 + w] if hb is not None else None
            if out_ap is None:
                hloc = hpool.tile([128, w], f32, tag=f"h{g}")
                out_ap = hloc[:]
            init = 0.0 if prev_ap is None else prev_ap
            nc.vector.tensor_tensor_scan(out_ap, av[:], bv[:], init,
                                         OP.mult, OP.add)
            nc.sync.dma_start(
                hout[g * 128:(g + 1) * 128, t * _TT + c0: t * _TT + c0 + w],
                out_ap)
            return out_ap[:, w - 1:w]

        def body(first, in_loop):
          hprev = [None] * _NG
          for t in range(_NT):
            if in_loop and staggered and t in (2, 4, 6):
                tc.stage_boundary()
            if first and t == 0:
                xs = xs0
            elif first and t == 1:
                xs = xs1
            else:
                xs = load_x(t)
            for g in range(_NG):
                last = (t == _NT - 1 and g == _NG - 1)
                prev_ap = None if t == 0 else hprev[g][:, _TT - 1:_TT]
                if split_last and last:
                    # old-style: two half-width MM pipelines for the drain
                    halves = ((0, _TT // 2), (_TT // 2, _TT // 2))
                    for (c0, w) in halves:
                        pz = plast.tile([128, w], f32, tag="pzl")
                        ph = plast.tile([128, w], f32, tag="phl")
                        for k in range(_NK):
                            nc.tensor.matmul(
                                pz[:],
                                lhsT=wz_sb[:, k * _EH + g * 128: k * _EH + (g + 1) * 128],
                                rhs=xs[:, k * _TT + c0: k * _TT + c0 + w],
                                start=(k == 0), stop=(k == _NK - 1))
                        for k in range(_NK):
                            nc.tensor.matmul(
                                ph[:],
                                lhsT=wh_sb[:, k * _EH + g * 128: k * _EH + (g + 1) * 128],
                                rhs=xs[:, k * _TT + c0: k * _TT + c0 + w],
                                start=(k == 0), stop=(k == _NK - 1))
                        prev_ap = epilogue(pz, ph, None, g, t, c0, w, prev_ap)
                    continue
                pz = ppool.tile([128, _TT], f32, tag="pz")
                ph = ppool.tile([128, _TT], f32, tag="ph")
                for k in range(_NK):
                    nc.tensor.matmul(
                        pz[:],
                        lhsT=wz_sb[:, k * _EH + g * 128: k * _EH + (g + 1) * 128],
                        rhs=xs[:, k * _TT:(k + 1) * _TT],
                        start=(k == 0),
                        stop=(k == _NK - 1),
                    )
                for k in range(_NK):
                    nc.tensor.matmul(
                        ph[:],
                        lhsT=wh_sb[:, k * _EH + g * 128: k * _EH + (g + 1) * 128],
                        rhs=xs[:, k * _TT:(k + 1) * _TT],
                        start=(k == 0),
                        stop=(k == _NK - 1),
                    )
                # Final group drains its epilogue in slices so the
                # ACT/DVE/DMA chain overlaps itself instead of running
                # full-width after the last matmul.
                slices = tuple((c, _LASTW) for c in range(0, _TT, _LASTW)) \
                    if last else ((0, _TT),)
                hb = hpool.tile([128, _TT], f32, tag=f"h{g}")
                for (c0, w) in slices:
                    prev_ap = epilogue(pz, ph, hb, g, t, c0, w, prev_ap)
                hprev[g] = hb

        if loop_n is not None:
            assert loop_n % unroll == 0
            body(True, False)
            from concourse import mybir as _mb
            with tc.For_i(0, loop_n // unroll, 1,
                          hint_engines=(_mb.EngineType.PE, _mb.EngineType.SP,
                                        _mb.EngineType.DVE,
                                        _mb.EngineType.Activation),
                          staggered_reset=staggered):
                for _u in range(unroll):
                    body(False, True)
        else:
            for rep in range(reps):
                body(rep == 0, False)
    nc.compile()
    return nc


_NC_CACHE = None


def _shard_inputs(inputs):
    """Per-core input maps: host pre-transposes so device DMAs are contiguous."""
    import ml_dtypes
    bf16 = ml_dtypes.bfloat16

    x = np.asarray(inputs["x"], dtype=np.float32)
    Wz = np.asarray(inputs["Wz"], dtype=np.float32)
    bz = np.asarray(inputs["bz"], dtype=np.float32)
    Wh = np.asarray(inputs["Wh"], dtype=np.float32)
    bh = np.asarray(inputs["bh"], dtype=np.float32)

    wzT = np.ascontiguousarray(Wz.T).astype(bf16)  # [d, e]
    whT = np.ascontiguousarray(Wh.T).astype(bf16)

    in_maps = []
    for i in range(8):
        b, j = i // 2, i % 2
        sl = slice(j * _EH, (j + 1) * _EH)
        in_maps.append({
            "xt": np.ascontiguousarray(x[b].T).astype(bf16),  # [D, T]
            "wzt": np.ascontiguousarray(wzT[:, sl]),          # [D, EH]
            "wht": np.ascontiguousarray(whT[:, sl]),          # [D, EH]
            "bzt": np.ascontiguousarray(bz[sl].reshape(_NG, 128).T),  # [128, NG]
            "nbzt": np.ascontiguousarray((-bz[sl]).reshape(_NG, 128).T),
            "bht": np.ascontiguousarray(bh[sl].reshape(_NG, 128).T),
        })
    return in_maps


def run(inputs, trace=False, tmpdir=None):
    """Shard inputs, run the SPMD kernel on 8 cores, reassemble full output.

    Returns (output ndarray (B, T, D) float32, BassKernelResults).
    """
    global _NC_CACHE
    from concourse.bass_utils import run_bass_kernel_spmd

    if _NC_CACHE is None:
        _NC_CACHE = _build()
    nc = _NC_CACHE

    in_maps = _shard_inputs(inputs)

    res = run_bass_kernel_spmd(
        nc, in_maps, core_ids=list(range(8)), trace=trace, tmpdir=tmpdir
    )

    out = np.empty((_B, _T, _D), dtype=np.float32)
    for i in range(8):
        b, j = i // 2, i % 2
        out[b, :, j * _EH:(j + 1) * _EH] = res.results[i]["h"].T
    return out, res


def kernel(**inputs):
    out, _ = run(inputs, trace=False)
    return out


# revision 26
# speedup vs baseline: 1.5226x; 1.0484x over previous
"""MinGRU (B=4, T=4096, D=1024) Trainium2 kernel, 8-core SPMD.

Sharding: core i handles (batch b = i//2, output-channel half j = i%2).
Each core computes u_z = x[b] @ Wz[half].T, u_h = x[b] @ Wh[half].T, then per
128-channel group: z = sigmoid(u_z + bz), a = 1 - z + eps, b = z*(u_h + bh),
and the recurrence h_t = a_t*h_{t-1} + b_t via the hardware
tensor_tensor_scan (fp32 PSUM/epilogue).

Matmul precision: h-path and the low `8-fp8k` contraction tiles of the
z-path run bf16 (1 col/cycle).  The top `fp8k` z-path k-tiles run fp8-e4m3
with perf_mode=DoubleRow (2 k-tiles per pass, ~1.4x) — sigmoid's <=1/4 slope
damps the extra quantization error; with fp8k=2 the end-to-end rel err is
1.37e-2 vs the 2e-2 gate (bf16-only: 2.4e-3).  Both z parts accumulate into
one PSUM at 32x scale (host pre-scales Wz by 32, exact for bf16; it lifts
the fp8 weights out of the subnormal range), undone by the sigmoid ACT's
scale=1/32.

Host pre-transposes x and W (and casts to bf16) so every DMA is
row-contiguous, and re-transposes the per-core (512, 4096) results into the
full output.  All DMAs ride the SP HWDGE queue in explicit priority order
(h-store dma_starts on the ACT queue would head-of-line-block the epilogue
ACTs; measured on HW: staggered_reset loop pipelining is a big net loss, so
the loop keeps the plain barrier back-edge, amortized via unroll).
"""

import numpy as np

_B, _T, _D = 4, 4096, 1024
_EH = 512          # output channels per core
_NG = _EH // 128   # 4 channel groups of 128 partitions
_TT = 512          # timestep tile (one PSUM bank, max moving free dim)
_NT = _T // _TT    # 8 t-tiles
_NK = _D // 128    # 8 contraction tiles


def _build(reps=1, loop_n=None, staggered=False, _LASTW=256,
           av_from_z=True, psum_bufs=3, split_last=True, unroll=2, fp8k=2):
    from contextlib import ExitStack
    from concourse import bacc, mybir, tile

    f32 = mybir.dt.float32
    bf16 = mybir.dt.bfloat16
    AF = mybir.ActivationFunctionType
    OP = mybir.AluOpType

    f8 = mybir.dt.float8e4
    DR = mybir.MatmulPerfMode.DoubleRow
    nbf = _NK - fp8k   # leading k-tiles of the z-path that stay bf16

    nc = bacc.Bacc("TRN2", debug=False, num_devices=8)
    xt = nc.dram_tensor("xt", [_D, _T], bf16, kind="ExternalInput").ap()
    xt8 = nc.dram_tensor("xt8", [_D, _T], f8, kind="ExternalInput").ap()
    wzt = nc.dram_tensor("wzt", [_D, _EH], bf16, kind="ExternalInput").ap()
    wz8t = nc.dram_tensor("wz8t", [_D, _EH], f8, kind="ExternalInput").ap()
    wht = nc.dram_tensor("wht", [_D, _EH], bf16, kind="ExternalInput").ap()
    bzt = nc.dram_tensor("bzt", [128, _NG], f32, kind="ExternalInput").ap()
    nbzt = nc.dram_tensor("nbzt", [128, _NG], f32, kind="ExternalInput").ap()
    bht = nc.dram_tensor("bht", [128, _NG], f32, kind="ExternalInput").ap()
    hout = nc.dram_tensor("h", [_EH, _T], f32, kind="ExternalOutput").ap()

    with tile.TileContext(nc) as tc, ExitStack() as ctx:
        wpool = ctx.enter_context(tc.tile_pool(name="w", bufs=1))
        xpool = ctx.enter_context(tc.tile_pool(name="x", bufs=3))
        vpool = ctx.enter_context(tc.tile_pool(name="v", bufs=3))
        hpool = ctx.enter_context(tc.tile_pool(name="h", bufs=2))
        ppool = ctx.enter_context(
            tc.tile_pool(name="p", bufs=psum_bufs, space="PSUM"))
        plast = ctx.enter_context(
            tc.tile_pool(name="pl", bufs=1, space="PSUM")) if split_last \
            else None

        def load_x(t, n_chunks=2):
            # x^T tile for this t-slice: [128, (k tt)], chunked k-block DMAs.
            xs = xpool.tile([128, _NK * _TT], bf16, tag="x")
            step = _NK // n_chunks
            for c in range(n_chunks):
                ks = c * step
                nc.sync.dma_start(
                    xs[:, ks * _TT:(ks + step) * _TT].rearrange(
                        "p (k t) -> p k t", k=step),
                    xt.rearrange("(k p) t -> p k t", p=128)[
                        :, ks:ks + step, t * _TT:(t + 1) * _TT],
                )
            return xs, load_x8(t)

        def load_x8(t):
            # fp8 copy of the top fp8k k-planes for the DoubleRow z matmuls
            if fp8k == 0:
                return None
            x8 = xpool.tile([128, fp8k * _TT], f8, tag="x8")
            nc.sync.dma_start(
                x8[:].rearrange("p (k t) -> p k t", k=fp8k),
                xt8.rearrange("(k p) t -> p k t", p=128)[
                    :, nbf:, t * _TT:(t + 1) * _TT],
            )
            return x8

        # Startup DMA order matters: the first matmuls need x chunk0 and the
        # first weight k-tiles; interleave x chunks with per-k weight tiles
        # so PE starts ~2us in and stays fed while the rest streams.
        xs0 = xpool.tile([128, _NK * _TT], bf16, tag="x")
        wz_sb = wpool.tile([128, _NK * _EH], bf16, tag="wz")
        wz8_sb = wpool.tile([128, max(fp8k, 1) * _EH], f8, tag="wz8")
        wh_sb = wpool.tile([128, _NK * _EH], bf16, tag="wh")
        bz_sb = wpool.tile([128, _NG], f32, tag="bz")
        nbz_sb = wpool.tile([128, _NG], f32, tag="nbz")
        bh_sb = wpool.tile([128, _NG], f32, tag="bh")

        def x0_chunk(ks, nk, eng):
            eng.dma_start(
                xs0[:, ks * _TT:(ks + nk) * _TT].rearrange(
                    "p (k t) -> p k t", k=nk),
                xt.rearrange("(k p) t -> p k t", p=128)[
                    :, ks:ks + nk, 0:_TT],
            )

        def w_chunk(k, eng):
            eng.dma_start(
                wz_sb[:, k * _EH:(k + 1) * _EH],
                wzt[k * 128:(k + 1) * 128, :],
            )
            eng.dma_start(
                wh_sb[:, k * _EH:(k + 1) * _EH],
                wht[k * 128:(k + 1) * 128, :],
            )

        # Startup rides ONE queue (sync) in consumption order: the two HWDGE
        # queues round-robin on shared HBM bandwidth, so split ordering lets
        # x chunks starve the per-k weight tiles the t=0 matmuls (k-major)
        # need.  x for t=1 is prefetched here too, between late weight tiles.
        w_chunk(0, nc.sync)
        x0_chunk(0, 2, nc.sync)
        if fp8k:
            # tiny fp8 tiles, but the DoubleRow pass closes every z
            # accumulation group, so they are needed almost immediately
            nc.sync.dma_start(
                wz8_sb[:, :fp8k * _EH].rearrange("p (k e) -> p k e", k=fp8k),
                wz8t.rearrange("(k p) e -> p k e", p=128)[:, nbf:, :])
        x8s0 = load_x8(0)
        w_chunk(1, nc.sync)
        nc.sync.dma_start(bz_sb[:], bzt)
        nc.sync.dma_start(nbz_sb[:], nbzt)
        nc.sync.dma_start(bh_sb[:], bht)
        x0_chunk(2, 2, nc.sync)
        w_chunk(2, nc.sync)
        w_chunk(3, nc.sync)
        x0_chunk(4, 2, nc.sync)
        w_chunk(4, nc.sync)
        x0_chunk(6, 2, nc.sync)
        w_chunk(5, nc.sync)
        w_chunk(6, nc.sync)
        w_chunk(7, nc.sync)
        xs1, x8s1 = load_x(1)

        def zmm(pz, xs, x8s, g, c0, w):
            # z-path contraction: nbf bf16 k-tiles + fp8k/2 DoubleRow passes
            npairs = fp8k // 2
            for k in range(nbf):
                nc.tensor.matmul(
                    pz[:],
                    lhsT=wz_sb[:, k * _EH + g * 128: k * _EH + (g + 1) * 128],
                    rhs=xs[:, k * _TT + c0: k * _TT + c0 + w],
                    start=(k == 0),
                    stop=(npairs == 0 and k == nbf - 1),
                )
            if npairs:
                v8w = wz8_sb[:, :fp8k * _EH].rearrange(
                    "p (k e) -> p k e", k=fp8k)
                v8x = x8s[:].rearrange("p (k t) -> p k t", k=fp8k)
                for p in range(npairs):
                    nc.tensor.matmul(
                        pz[:],
                        lhsT=v8w[:, 2 * p:2 * p + 2, g * 128:(g + 1) * 128],
                        rhs=v8x[:, 2 * p:2 * p + 2, c0:c0 + w],
                        start=(nbf == 0 and p == 0),
                        stop=(p == npairs - 1),
                        perf_mode=DR,
                    )

        def epilogue(pz, ph, hb, g, t, c0, w, prev_ap):
            z = vpool.tile([128, w], f32, tag="z")
            nc.scalar.activation(z[:], pz[:, c0 - c0:c0 - c0 + w] if hb is None
                                 else pz[:, c0:c0 + w], AF.Sigmoid,
                                 bias=bz_sb[:, g:g + 1], scale=1.0 / 32.0)
            av = vpool.tile([128, w], f32, tag="a")
            if av_from_z:
                # old chain: a = (1+eps) - z on ACT Copy
                nc.scalar.activation(av[:], z[:], AF.Copy,
                                     bias=1.0 + 1e-8, scale=-1.0)
            else:
                # a = 1 - z == sigmoid(-(u+bz)); independent of z's ACT.
                nc.scalar.activation(av[:], pz[:, c0:c0 + w] if hb is not None
                                     else pz[:], AF.Sigmoid,
                                     bias=nbz_sb[:, g:g + 1], scale=-1.0 / 32.0)
            bv = vpool.tile([128, w], f32, tag="b")
            nc.vector.scalar_tensor_tensor(
                bv[:], ph[:, c0:c0 + w] if hb is not None else ph[:],
                bh_sb[:, g:g + 1], z[:], OP.add, OP.mult)
            out_ap = hb[:, c0:c0 + w] if hb is not None else None
            if out_ap is None:
                hloc = hpool.tile([128, w], f32, tag=f"h{g}")
                out_ap = hloc[:]
            init = 0.0 if prev_ap is None else prev_ap
            nc.vector.tensor_tensor_scan(out_ap, av[:], bv[:], init,
                                         OP.mult, OP.add)
            nc.sync.dma_start(
                hout[g * 128:(g + 1) * 128, t * _TT + c0: t * _TT + c0 + w],
                out_ap)
            return out_ap[:, w - 1:w]

        def body(first, in_loop):
          hprev = [None] * _NG
          for t in range(_NT):
            if in_loop and staggered and t in (2, 4, 6):
                tc.stage_boundary()
            if first and t == 0:
                xs, x8s = xs0, x8s0
            elif first and t == 1:
                xs, x8s = xs1, x8s1
            else:
                xs, x8s = load_x(t)
            for g in range(_NG):
                last = (t == _NT - 1 and g == _NG - 1)
                prev_ap = None if t == 0 else hprev[g][:, _TT - 1:_TT]
                if split_last and last:
                    # old-style: two half-width MM pipelines for the drain
                    halves = ((0, _TT // 2), (_TT // 2, _TT // 2))
                    for (c0, w) in halves:
                        pz = plast.tile([128, w], f32, tag="pzl")
                        ph = plast.tile([128, w], f32, tag="phl")
                        zmm(pz, xs, x8s, g, c0, w)
                        for k in range(_NK):
                            nc.tensor.matmul(
                                ph[:],
                                lhsT=wh_sb[:, k * _EH + g * 128: k * _EH + (g + 1) * 128],
                                rhs=xs[:, k * _TT + c0: k * _TT + c0 + w],
                                start=(k == 0), stop=(k == _NK - 1))
                        prev_ap = epilogue(pz, ph, None, g, t, c0, w, prev_ap)
                    continue
                pz = ppool.tile([128, _TT], f32, tag="pz")
                ph = ppool.tile([128, _TT], f32, tag="ph")
                zmm(pz, xs, x8s, g, 0, _TT)
                for k in range(_NK):
                    nc.tensor.matmul(
                        ph[:],
                        lhsT=wh_sb[:, k * _EH + g * 128: k * _EH + (g + 1) * 128],
                        rhs=xs[:, k * _TT:(k + 1) * _TT],
                        start=(k == 0),
                        stop=(k == _NK - 1),
                    )
                # Final group drains its epilogue in slices so the
                # ACT/DVE/DMA chain overlaps itself instead of running
                # full-width after the last matmul.
                slices = tuple((c, _LASTW) for c in range(0, _TT, _LASTW)) \
                    if last else ((0, _TT),)
                hb = hpool.tile([128, _TT], f32, tag=f"h{g}")
                for (c0, w) in slices:
                    prev_ap = epilogue(pz, ph, hb, g, t, c0, w, prev_ap)
                hprev[g] = hb

        if loop_n is not None:
            assert loop_n % unroll == 0
            body(True, False)
            from concourse import mybir as _mb
            with tc.For_i(0, loop_n // unroll, 1,
                          hint_engines=(_mb.EngineType.PE, _mb.EngineType.SP,
                                        _mb.EngineType.DVE,
                                        _mb.EngineType.Activation),
                          staggered_reset=staggered):
                for _u in range(unroll):
                    body(False, True)
        else:
            for rep in range(reps):
                body(rep == 0, False)
    nc.compile()
    return nc


_NC_CACHE = None


def _shard_inputs(inputs):
    """Per-core input maps: host pre-transposes so device DMAs are contiguous."""
    import ml_dtypes
    bf16 = ml_dtypes.bfloat16

    x = np.asarray(inputs["x"], dtype=np.float32)
    Wz = np.asarray(inputs["Wz"], dtype=np.float32)
    bz = np.asarray(inputs["bz"], dtype=np.float32)
    Wh = np.asarray(inputs["Wh"], dtype=np.float32)
    bh = np.asarray(inputs["bh"], dtype=np.float32)

    import ml_dtypes as _mld
    e4 = _mld.float8_e4m3
    wzT32 = np.ascontiguousarray(Wz.T) * 32.0      # [d, e], x32 (see _build)
    wzT = wzT32.astype(bf16)
    wzT8 = wzT32.astype(e4)
    whT = np.ascontiguousarray(Wh.T).astype(bf16)

    in_maps = []
    for i in range(8):
        b, j = i // 2, i % 2
        sl = slice(j * _EH, (j + 1) * _EH)
        xT = np.ascontiguousarray(x[b].T)
        in_maps.append({
            "xt": xT.astype(bf16),                            # [D, T]
            "xt8": xT.astype(e4),                             # [D, T] fp8
            "wzt": np.ascontiguousarray(wzT[:, sl]),          # [D, EH] x32
            "wz8t": np.ascontiguousarray(wzT8[:, sl]),        # [D, EH] x32
            "wht": np.ascontiguousarray(whT[:, sl]),          # [D, EH]
            "bzt": np.ascontiguousarray(bz[sl].reshape(_NG, 128).T),  # [128, NG]
            "nbzt": np.ascontiguousarray((-bz[sl]).reshape(_NG, 128).T),
            "bht": np.ascontiguousarray(bh[sl].reshape(_NG, 128).T),
        })
    return in_maps


def run(inputs, trace=False, tmpdir=None):
    """Shard inputs, run the SPMD kernel on 8 cores, reassemble full output.

    Returns (output ndarray (B, T, D) float32, BassKernelResults).
    """
    global _NC_CACHE
    from concourse.bass_utils import run_bass_kernel_spmd

    if _NC_CACHE is None:
        _NC_CACHE = _build()
    nc = _NC_CACHE

    in_maps = _shard_inputs(inputs)

    res = run_bass_kernel_spmd(
        nc, in_maps, core_ids=list(range(8)), trace=trace, tmpdir=tmpdir
    )

    out = np.empty((_B, _T, _D), dtype=np.float32)
    for i in range(8):
        b, j = i // 2, i % 2
        out[b, :, j * _EH:(j + 1) * _EH] = res.results[i]["h"].T
    return out, res


def kernel(**inputs):
    out, _ = run(inputs, trace=False)
    return out


# revision 27
# speedup vs baseline: 1.5263x; 1.0024x over previous
"""MinGRU (B=4, T=4096, D=1024) Trainium2 kernel, 8-core SPMD.

Sharding: core i handles (batch b = i//2, output-channel half j = i%2).
Each core computes u_z = x[b] @ Wz[half].T, u_h = x[b] @ Wh[half].T, then per
128-channel group: z = sigmoid(u_z + bz), a = 1 - z + eps, b = z*(u_h + bh),
and the recurrence h_t = a_t*h_{t-1} + b_t via the hardware
tensor_tensor_scan (fp32 PSUM/epilogue).

Matmul precision: h-path and the low `8-fp8k` contraction tiles of the
z-path run bf16 (1 col/cycle).  The top `fp8k` z-path k-tiles run fp8-e4m3
with perf_mode=DoubleRow (2 k-tiles per pass, ~1.4x) — sigmoid's <=1/4 slope
damps the extra quantization error; with fp8k=2 the end-to-end rel err is
1.37e-2 vs the 2e-2 gate (bf16-only: 2.4e-3).  Both z parts accumulate into
one PSUM at 32x scale (host pre-scales Wz by 32, exact for bf16; it lifts
the fp8 weights out of the subnormal range), undone by the sigmoid ACT's
scale=1/32.

Host pre-transposes x and W (and casts to bf16) so every DMA is
row-contiguous, and re-transposes the per-core (512, 4096) results into the
full output.  All DMAs ride the SP HWDGE queue in explicit priority order
(h-store dma_starts on the ACT queue would head-of-line-block the epilogue
ACTs; measured on HW: staggered_reset loop pipelining is a big net loss, so
the loop keeps the plain barrier back-edge, amortized via unroll).
"""

import numpy as np

_B, _T, _D = 4, 4096, 1024
_EH = 512          # output channels per core
_NG = _EH // 128   # 4 channel groups of 128 partitions
_TT = 512          # timestep tile (one PSUM bank, max moving free dim)
_NT = _T // _TT    # 8 t-tiles
_NK = _D // 128    # 8 contraction tiles


def _build(reps=1, loop_n=None, staggered=False, _LASTW=256,
           av_from_z=True, psum_bufs=3, split_last=True, unroll=4, fp8k=2):
    from contextlib import ExitStack
    from concourse import bacc, mybir, tile

    f32 = mybir.dt.float32
    bf16 = mybir.dt.bfloat16
    AF = mybir.ActivationFunctionType
    OP = mybir.AluOpType

    f8 = mybir.dt.float8e4
    DR = mybir.MatmulPerfMode.DoubleRow
    nbf = _NK - fp8k   # leading k-tiles of the z-path that stay bf16

    nc = bacc.Bacc("TRN2", debug=False, num_devices=8)
    xt = nc.dram_tensor("xt", [_D, _T], bf16, kind="ExternalInput").ap()
    xt8 = nc.dram_tensor("xt8", [_D, _T], f8, kind="ExternalInput").ap()
    wzt = nc.dram_tensor("wzt", [_D, _EH], bf16, kind="ExternalInput").ap()
    wz8t = nc.dram_tensor("wz8t", [_D, _EH], f8, kind="ExternalInput").ap()
    wht = nc.dram_tensor("wht", [_D, _EH], bf16, kind="ExternalInput").ap()
    bzt = nc.dram_tensor("bzt", [128, _NG], f32, kind="ExternalInput").ap()
    nbzt = nc.dram_tensor("nbzt", [128, _NG], f32, kind="ExternalInput").ap()
    bht = nc.dram_tensor("bht", [128, _NG], f32, kind="ExternalInput").ap()
    hout = nc.dram_tensor("h", [_EH, _T], f32, kind="ExternalOutput").ap()

    with tile.TileContext(nc) as tc, ExitStack() as ctx:
        wpool = ctx.enter_context(tc.tile_pool(name="w", bufs=1))
        xpool = ctx.enter_context(tc.tile_pool(name="x", bufs=3))
        vpool = ctx.enter_context(tc.tile_pool(name="v", bufs=3))
        hpool = ctx.enter_context(tc.tile_pool(name="h", bufs=2))
        ppool = ctx.enter_context(
            tc.tile_pool(name="p", bufs=psum_bufs, space="PSUM"))
        plast = ctx.enter_context(
            tc.tile_pool(name="pl", bufs=1, space="PSUM")) if split_last \
            else None

        def load_x(t, n_chunks=2):
            # x^T tile for this t-slice: [128, (k tt)], chunked k-block DMAs.
            xs = xpool.tile([128, _NK * _TT], bf16, tag="x")
            step = _NK // n_chunks
            for c in range(n_chunks):
                ks = c * step
                nc.sync.dma_start(
                    xs[:, ks * _TT:(ks + step) * _TT].rearrange(
                        "p (k t) -> p k t", k=step),
                    xt.rearrange("(k p) t -> p k t", p=128)[
                        :, ks:ks + step, t * _TT:(t + 1) * _TT],
                )
            return xs, load_x8(t)

        def load_x8(t):
            # fp8 copy of the top fp8k k-planes for the DoubleRow z matmuls
            if fp8k == 0:
                return None
            x8 = xpool.tile([128, fp8k * _TT], f8, tag="x8")
            nc.sync.dma_start(
                x8[:].rearrange("p (k t) -> p k t", k=fp8k),
                xt8.rearrange("(k p) t -> p k t", p=128)[
                    :, nbf:, t * _TT:(t + 1) * _TT],
            )
            return x8

        # Startup DMA order matters: the first matmuls need x chunk0 and the
        # first weight k-tiles; interleave x chunks with per-k weight tiles
        # so PE starts ~2us in and stays fed while the rest streams.
        xs0 = xpool.tile([128, _NK * _TT], bf16, tag="x")
        wz_sb = wpool.tile([128, _NK * _EH], bf16, tag="wz")
        wz8_sb = wpool.tile([128, max(fp8k, 1) * _EH], f8, tag="wz8")
        wh_sb = wpool.tile([128, _NK * _EH], bf16, tag="wh")
        bz_sb = wpool.tile([128, _NG], f32, tag="bz")
        nbz_sb = wpool.tile([128, _NG], f32, tag="nbz")
        bh_sb = wpool.tile([128, _NG], f32, tag="bh")

        def x0_chunk(ks, nk, eng):
            eng.dma_start(
                xs0[:, ks * _TT:(ks + nk) * _TT].rearrange(
                    "p (k t) -> p k t", k=nk),
                xt.rearrange("(k p) t -> p k t", p=128)[
                    :, ks:ks + nk, 0:_TT],
            )

        def w_chunk(k, eng):
            eng.dma_start(
                wz_sb[:, k * _EH:(k + 1) * _EH],
                wzt[k * 128:(k + 1) * 128, :],
            )
            eng.dma_start(
                wh_sb[:, k * _EH:(k + 1) * _EH],
                wht[k * 128:(k + 1) * 128, :],
            )

        # Startup rides ONE queue (sync) in consumption order: the two HWDGE
        # queues round-robin on shared HBM bandwidth, so split ordering lets
        # x chunks starve the per-k weight tiles the t=0 matmuls (k-major)
        # need.  x for t=1 is prefetched here too, between late weight tiles.
        w_chunk(0, nc.sync)
        x0_chunk(0, 2, nc.sync)
        if fp8k:
            # tiny fp8 tiles, but the DoubleRow pass closes every z
            # accumulation group, so they are needed almost immediately
            nc.sync.dma_start(
                wz8_sb[:, :fp8k * _EH].rearrange("p (k e) -> p k e", k=fp8k),
                wz8t.rearrange("(k p) e -> p k e", p=128)[:, nbf:, :])
        x8s0 = load_x8(0)
        w_chunk(1, nc.sync)
        nc.sync.dma_start(bz_sb[:], bzt)
        nc.sync.dma_start(nbz_sb[:], nbzt)
        nc.sync.dma_start(bh_sb[:], bht)
        x0_chunk(2, 2, nc.sync)
        w_chunk(2, nc.sync)
        w_chunk(3, nc.sync)
        x0_chunk(4, 2, nc.sync)
        w_chunk(4, nc.sync)
        x0_chunk(6, 2, nc.sync)
        w_chunk(5, nc.sync)
        w_chunk(6, nc.sync)
        w_chunk(7, nc.sync)
        xs1, x8s1 = load_x(1)

        def zmm(pz, xs, x8s, g, c0, w):
            # z-path contraction: nbf bf16 k-tiles + fp8k/2 DoubleRow passes
            npairs = fp8k // 2
            for k in range(nbf):
                nc.tensor.matmul(
                    pz[:],
                    lhsT=wz_sb[:, k * _EH + g * 128: k * _EH + (g + 1) * 128],
                    rhs=xs[:, k * _TT + c0: k * _TT + c0 + w],
                    start=(k == 0),
                    stop=(npairs == 0 and k == nbf - 1),
                )
            if npairs:
                v8w = wz8_sb[:, :fp8k * _EH].rearrange(
                    "p (k e) -> p k e", k=fp8k)
                v8x = x8s[:].rearrange("p (k t) -> p k t", k=fp8k)
                for p in range(npairs):
                    nc.tensor.matmul(
                        pz[:],
                        lhsT=v8w[:, 2 * p:2 * p + 2, g * 128:(g + 1) * 128],
                        rhs=v8x[:, 2 * p:2 * p + 2, c0:c0 + w],
                        start=(nbf == 0 and p == 0),
                        stop=(p == npairs - 1),
                        perf_mode=DR,
                    )

        def epilogue(pz, ph, hb, g, t, c0, w, prev_ap):
            z = vpool.tile([128, w], f32, tag="z")
            nc.scalar.activation(z[:], pz[:, c0 - c0:c0 - c0 + w] if hb is None
                                 else pz[:, c0:c0 + w], AF.Sigmoid,
                                 bias=bz_sb[:, g:g + 1], scale=1.0 / 32.0)
            av = vpool.tile([128, w], f32, tag="a")
            if av_from_z:
                # old chain: a = (1+eps) - z on ACT Copy
                nc.scalar.activation(av[:], z[:], AF.Copy,
                                     bias=1.0 + 1e-8, scale=-1.0)
            else:
                # a = 1 - z == sigmoid(-(u+bz)); independent of z's ACT.
                nc.scalar.activation(av[:], pz[:, c0:c0 + w] if hb is not None
                                     else pz[:], AF.Sigmoid,
                                     bias=nbz_sb[:, g:g + 1], scale=-1.0 / 32.0)
            bv = vpool.tile([128, w], f32, tag="b")
            nc.vector.scalar_tensor_tensor(
                bv[:], ph[:, c0:c0 + w] if hb is not None else ph[:],
                bh_sb[:, g:g + 1], z[:], OP.add, OP.mult)
            out_ap = hb[:, c0:c0 + w] if hb is not None else None
            if out_ap is None:
                hloc = hpool.tile([128, w], f32, tag=f"h{g}")
                out_ap = hloc[:]
            init = 0.0 if prev_ap is None else prev_ap
            nc.vector.tensor_tensor_scan(out_ap, av[:], bv[:], init,
                                         OP.mult, OP.add)
            nc.sync.dma_start(
                hout[g * 128:(g + 1) * 128, t * _TT + c0: t * _TT + c0 + w],
                out_ap)
            return out_ap[:, w - 1:w]

        def body(first, in_loop):
          hprev = [None] * _NG
          for t in range(_NT):
            if in_loop and staggered and t in (2, 4, 6):
                tc.stage_boundary()
            if first and t == 0:
                xs, x8s = xs0, x8s0
            elif first and t == 1:
                xs, x8s = xs1, x8s1
            else:
                xs, x8s = load_x(t)
            for g in range(_NG):
                last = (t == _NT - 1 and g == _NG - 1)
                prev_ap = None if t == 0 else hprev[g][:, _TT - 1:_TT]
                if split_last and last:
                    # old-style: two half-width MM pipelines for the drain
                    halves = ((0, _TT // 2), (_TT // 2, _TT // 2))
                    for (c0, w) in halves:
                        pz = plast.tile([128, w], f32, tag="pzl")
                        ph = plast.tile([128, w], f32, tag="phl")
                        zmm(pz, xs, x8s, g, c0, w)
                        for k in range(_NK):
                            nc.tensor.matmul(
                                ph[:],
                                lhsT=wh_sb[:, k * _EH + g * 128: k * _EH + (g + 1) * 128],
                                rhs=xs[:, k * _TT + c0: k * _TT + c0 + w],
                                start=(k == 0), stop=(k == _NK - 1))
                        prev_ap = epilogue(pz, ph, None, g, t, c0, w, prev_ap)
                    continue
                pz = ppool.tile([128, _TT], f32, tag="pz")
                ph = ppool.tile([128, _TT], f32, tag="ph")
                zmm(pz, xs, x8s, g, 0, _TT)
                for k in range(_NK):
                    nc.tensor.matmul(
                        ph[:],
                        lhsT=wh_sb[:, k * _EH + g * 128: k * _EH + (g + 1) * 128],
                        rhs=xs[:, k * _TT:(k + 1) * _TT],
                        start=(k == 0),
                        stop=(k == _NK - 1),
                    )
                # Final group drains its epilogue in slices so the
                # ACT/DVE/DMA chain overlaps itself instead of running
                # full-width after the last matmul.
                slices = tuple((c, _LASTW) for c in range(0, _TT, _LASTW)) \
                    if last else ((0, _TT),)
                hb = hpool.tile([128, _TT], f32, tag=f"h{g}")
                for (c0, w) in slices:
                    prev_ap = epilogue(pz, ph, hb, g, t, c0, w, prev_ap)
                hprev[g] = hb

        if loop_n is not None:
            assert loop_n % unroll == 0
            body(True, False)
            from concourse import mybir as _mb
            with tc.For_i(0, loop_n // unroll, 1,
                          hint_engines=(_mb.EngineType.PE, _mb.EngineType.SP,
                                        _mb.EngineType.DVE,
                                        _mb.EngineType.Activation),
                          staggered_reset=staggered):
                for _u in range(unroll):
                    body(False, True)
        else:
            for rep in range(reps):
                body(rep == 0, False)
    nc.compile()
    return nc


_NC_CACHE = None


def _shard_inputs(inputs):
    """Per-core input maps: host pre-transposes so device DMAs are contiguous."""
    import ml_dtypes
    bf16 = ml_dtypes.bfloat16

    x = np.asarray(inputs["x"], dtype=np.float32)
    Wz = np.asarray(inputs["Wz"], dtype=np.float32)
    bz = np.asarray(inputs["bz"], dtype=np.float32)
    Wh = np.asarray(inputs["Wh"], dtype=np.float32)
    bh = np.asarray(inputs["bh"], dtype=np.float32)

    import ml_dtypes as _mld
    e4 = _mld.float8_e4m3
    wzT32 = np.ascontiguousarray(Wz.T) * 32.0      # [d, e], x32 (see _build)
    wzT = wzT32.astype(bf16)
    wzT8 = wzT32.astype(e4)
    whT = np.ascontiguousarray(Wh.T).astype(bf16)

    in_maps = []
    for i in range(8):
        b, j = i // 2, i % 2
        sl = slice(j * _EH, (j + 1) * _EH)
        xT = np.ascontiguousarray(x[b].T)
        in_maps.append({
            "xt": xT.astype(bf16),                            # [D, T]
            "xt8": xT.astype(e4),                             # [D, T] fp8
            "wzt": np.ascontiguousarray(wzT[:, sl]),          # [D, EH] x32
            "wz8t": np.ascontiguousarray(wzT8[:, sl]),        # [D, EH] x32
            "wht": np.ascontiguousarray(whT[:, sl]),          # [D, EH]
            "bzt": np.ascontiguousarray(bz[sl].reshape(_NG, 128).T),  # [128, NG]
            "nbzt": np.ascontiguousarray((-bz[sl]).reshape(_NG, 128).T),
            "bht": np.ascontiguousarray(bh[sl].reshape(_NG, 128).T),
        })
    return in_maps


def run(inputs, trace=False, tmpdir=None):
    """Shard inputs, run the SPMD kernel on 8 cores, reassemble full output.

    Returns (output ndarray (B, T, D) float32, BassKernelResults).
    """
    global _NC_CACHE
    from concourse.bass_utils import run_bass_kernel_spmd

    if _NC_CACHE is None:
        _NC_CACHE = _build()
    nc = _NC_CACHE

    in_maps = _shard_inputs(inputs)

    res = run_bass_kernel_spmd(
        nc, in_maps, core_ids=list(range(8)), trace=trace, tmpdir=tmpdir
    )

    out = np.empty((_B, _T, _D), dtype=np.float32)
    for i in range(8):
        b, j = i // 2, i % 2
        out[b, :, j * _EH:(j + 1) * _EH] = res.results[i]["h"].T
    return out, res


def kernel(**inputs):
    out, _ = run(inputs, trace=False)
    return out
